# revision 77
# baseline (speedup 1.0000x reference)
"""Causal self-attention (B=4, T=2048, C=1024, H=16, D=64) on 8 TRN2 cores.

Sharding: core = 2*b + hg  (b = batch 0..3, hg = head-group 0..1 of 8 heads).
Each core computes its batch's QKV projections for its 8 heads, RMSNorm+RoPE,
causal attention, and a partial output projection over its head-group's wproj
rows; the two partials per batch are summed on the host.

v3 pipeline:
  QKV projections run as split-high/low fp8e4m3 DoubleRow matmuls
  (x = xh + xl, w = wh + wl quantized on host; ps = xh@wh + xh@wl + xl@wh,
  ~0.3% rms error, 0.75x the fp16 matmul cost).  RMS scale invariance
  absorbs the w*16 fp8 range scaling for q/k; the v-blend divides by 16.
  RoPE+RMS: t1 = ps*cos, u = ps*sin (DVE, frees the PSUM quickly), RMS
  stats from t1^2+u^2 = ps^2 (DVE fp16), rot = [t1_0+u_1 | t1_1-u_0]
  (Pool), 1/rms applied on Pool, then feature-major transpose via the
  XBAR DMA-transpose engine (zero PE cost).
  Attention per (query chunk qc, head pair hp): S^T per head into one
  [128, 2, 512] PSUM tile, ONE merged exp for both heads (ACT), fp16
  triangle mask on diagonal blocks (DVE 4x), PV accumulates [65, W] per
  head up to 6 steps behind; the 65th ones-column of v computes softmax
  denominators in the same matmul.  Each head-pair's normalization
  (reciprocal -> ones-matmul broadcast -> fp16 yT) is deferred into the
  next pair's loop; output projections are deferred into later chunks'
  attention steps (proj 0 -> chunk 1, proj 1 and 2 -> chunk 3), as are
  the next chunk's QKV units, so PE always has fill work during
  exp-limited stretches.

The ISA has ONE semaphore-wait slot per instruction; Tile emits more.
_legalize_waits() splits extras onto same-engine NoOps post-scheduling.
DmaTransposeAnt cannot encode any wait - all its waits move to NoOps.
"""

import math

import numpy as np
import ml_dtypes

import concourse.bass as bass
import concourse.mybir as mybir
import concourse.tile as tile
from concourse import bass_utils

F32 = mybir.dt.float32
F16 = mybir.dt.float16
F8 = mybir.dt.float8e4

B, T, C, H, D = 4, 2048, 1024, 16, 64
HG = C // 2          # 512 features per head group (8 heads x 64)
NT = T // 128        # 16 t-tiles
NQ = T // 512        # 4 query/t chunks
EPS = 1.1920928955078125e-07
SCALE = 1.0 / math.sqrt(D)  # 0.125

_wsplit_counter = [0]


def _legalize_waits(nc):
    """Split multi-wait instructions into single-wait NoOp chains."""
    n = 0
    for f in nc.m.functions:
        for bb in f.blocks:
            new_list = []
            changed = False
            for inst in bb.instructions:
                si = inst.sync_info
                is_dt = type(inst).__name__ == "InstDmaTransposeAnt"
                keep = 0 if is_dt else 1
                if si is not None and si.on_wait and len(si.on_wait) > keep:
                    waits = list(si.on_wait)
                    for w in (waits if is_dt else waits[:-1]):
                        _wsplit_counter[0] += 1
                        new_list.append(mybir.InstNoOp(
                            name=f"WSPLIT-{_wsplit_counter[0]}",
                            engine=inst.engine, ins=[], outs=[],
                            sync_info=mybir.SyncInfo(on_wait=[w], on_update=[]),
                        ))
                    si.on_wait = [] if is_dt else waits[-1:]
                    changed = True
                    n += 1
                new_list.append(inst)
            if changed:
                bb.instructions = new_list
    return n


def _build(lam: float) -> bass.Bass:
    nc = bass.Bass("TRN2", target_bir_lowering=False, debug=False,
                   num_devices=8)

    xh_d = nc.dram_tensor("xTh", [C, T], F8, kind="ExternalInput").ap()
    xl_d = nc.dram_tensor("xTl", [C, T], F8, kind="ExternalInput").ap()
    v1_d = nc.dram_tensor("v1h", [T, HG], F16, kind="ExternalInput").ap()
    w_d = {}
    for wn in ("q", "k", "v"):
        for piece in ("h", "l"):
            w_d[wn + piece] = nc.dram_tensor(
                f"w{wn}{piece}", [C, HG], F8, kind="ExternalInput").ap()
    wp_d = nc.dram_tensor("wpT", [HG, C], F16, kind="ExternalInput").ap()
    cs_d = nc.dram_tensor("csn", [T, 128], F16, kind="ExternalInput").ap()
    tri_d = nc.dram_tensor("tri01", [128, 128], F16, kind="ExternalInput").ap()
    out_d = nc.dram_tensor("out", [T, C], F16, kind="ExternalOutput").ap()

    with tile.TileContext(nc) as tc:
        with (
            tc.tile_pool(name="const", bufs=1) as const,
            tc.tile_pool(name="pers", bufs=1) as pers,
        ):
            tri01 = const.tile([128, 128], F16)
            nc.gpsimd.dma_start(out=tri01, in_=tri_d)
            ones81 = const.tile([128, 8, 1], F16)
            nc.vector.memset(ones81, 1.0)
            epsc = const.tile([128, 1], F32)
            nc.vector.memset(epsc, EPS)
            ones64 = const.tile([1, 64], F16)
            nc.vector.memset(ones64, 1.0)

            # persistent feature-major q/k and v tiles
            qT = pers.tile([128, 4, T], F16, name="qT", tag="qT")
            kT = pers.tile([128, 4, T], F16, name="kT", tag="kT")
            vsb = [pers.tile([128, 8, 65], F16, name=f"v{t}", tag=f"v{t}")
                   for t in range(NT)]

            with (
                tc.tile_pool(name="p1", bufs=1) as p1,
                tc.tile_pool(name="p1ps", bufs=1, space="PSUM") as p1ps,
                tc.tile_pool(name="p2", bufs=1) as p2,
            ):
                # chunk-0 x DMAs first: the very first matmul needs them
                x0h = p1.tile([128, 8, 512], F8, name="xh", tag="xh", bufs=2)
                x0l = p1.tile([128, 8, 512], F8, name="xl", tag="xl", bufs=2)
                x0hr = xh_d[:, 0:512].rearrange("(c p) t -> p c t", p=128)
                x0lr = xl_d[:, 0:512].rearrange("(c p) t -> p c t", p=128)
                nc.sync.dma_start(out=x0h[:, 0:2, :], in_=x0hr[:, 0:2, :])
                nc.sync.dma_start(out=x0l[:, 0:2, :], in_=x0lr[:, 0:2, :])
                nc.sync.dma_start(out=x0h[:, 2:8, :], in_=x0hr[:, 2:8, :])
                nc.sync.dma_start(out=x0l[:, 2:8, :], in_=x0lr[:, 2:8, :])

                w_sb = {}
                for i, wn in enumerate(("qh", "ql", "kh", "kl", "vh", "vl")):
                    w_sb[wn] = p1.tile([128, 8, HG], F8, name=f"w{wn}")
                    eng = (nc.scalar, nc.gpsimd)[i % 2]
                    wr = w_d[wn[0] + wn[1]].rearrange("(c p) i -> p c i",
                                                      p=128)
                    if wn in ("qh", "ql"):
                        eng.dma_start(out=w_sb[wn][:, 0:2, :],
                                      in_=wr[:, 0:2, :])
                        eng.dma_start(out=w_sb[wn][:, 2:8, :],
                                      in_=wr[:, 2:8, :])
                    else:
                        eng.dma_start(out=w_sb[wn], in_=wr)
                cs_sb = p1.tile([128, NT, 128], F16)
                nc.gpsimd.dma_start(
                    out=cs_sb, in_=cs_d.rearrange("(n p) i -> p n i", p=128))
                wp_sb = p2.tile([128, 4, C], F16)
                wp_loaded = [False]

                def p1_gen(tc4, xtiles=None, use_st=False):
                    if xtiles is not None:
                        xh, xl = xtiles
                    else:
                        xh = p1.tile([128, 8, 512], F8, name="xh", tag="xh",
                                     bufs=2)
                        xl = p1.tile([128, 8, 512], F8, name="xl", tag="xl",
                                     bufs=2)
                        t0 = tc4 * 512
                        nc.sync.dma_start(
                            out=xh, in_=xh_d[:, t0:t0 + 512].rearrange(
                                "(c p) t -> p c t", p=128))
                        nc.sync.dma_start(
                            out=xl, in_=xl_d[:, t0:t0 + 512].rearrange(
                                "(c p) t -> p c t", p=128))
                    for which in ("q", "k", "v"):
                        for ts in range(4):
                            tg = tc4 * 4 + ts
                            yield
                            wh = w_sb[which + "h"]
                            wl = w_sb[which + "l"]
                            if use_st and (ts % 2 == 0):
                                ps = p1ps.tile([128, 2, 512], F32, name="st",
                                               tag="st", bufs=2)[:, 0, :]
                            else:
                                ps = p1ps.tile([128, 512], F32, name="qkvps",
                                               tag="qkvps", bufs=2)
                            terms = ((xh, wh), (xh, wl), (xl, wh))
                            for ti, (xt, wt) in enumerate(terms):
                                for c in range(4):
                                    nc.tensor.matmul(
                                        ps,
                                        xt[:, 2 * c:2 * c + 2,
                                           ts * 128:(ts + 1) * 128],
                                        wt[:, 2 * c:2 * c + 2, :],
                                        start=(ti == 0 and c == 0),
                                        stop=(ti == 2 and c == 3),
                                        perf_mode=mybir.MatmulPerfMode
                                        .DoubleRow)
                            p3 = ps.rearrange("p (h d) -> p h d", h=8)
                            if which == "v":
                                v1t = p1.tile([128, HG], F16, name="v1t",
                                              tag="v1t", bufs=2)
                                nc.sync.dma_start(
                                    out=v1t,
                                    in_=v1_d[tg * 128:(tg + 1) * 128, :])
                                nc.vector.scalar_tensor_tensor(
                                    out=vsb[tg][:, :, 0:64],
                                    in0=p3,
                                    scalar=(1.0 - lam) / 16.0,
                                    in1=v1t.rearrange("p (h d) -> p h d", h=8),
                                    op0=mybir.AluOpType.mult,
                                    op1=mybir.AluOpType.add)
                                nc.vector.tensor_copy(
                                    out=vsb[tg][:, :, 64:65], in_=ones81)
                                continue

                            # ps freed by two quick DVE ops (t1, u); RMS
                            # stats from t1^2+u^2 = ps^2*(c^2+s^2) (scale
                            # invariant, so the 16x fp8 w-scale cancels).
                            cs3 = cs_sb[:, tg, :].rearrange(
                                "p (o cs dd) -> p o cs dd",
                                o=1, cs=2).to_broadcast((128, 8, 2, 64))
                            p5 = ps.rearrange(
                                "p (h dd) -> p h dd", h=8).rearrange(
                                "p h (o dd) -> p h o dd",
                                o=1).to_broadcast((128, 8, 2, 64))
                            tu = p1.tile([128, 8, 2, 64], F16, name="tu",
                                         tag="tu", bufs=2)
                            nc.vector.tensor_mul(out=tu, in0=p5, in1=cs3)
                            t1 = tu[:, :, 0, :].rearrange(
                                "p h (two d) -> p h two d", two=2)
                            u = tu[:, :, 1, :].rearrange(
                                "p h (two d) -> p h two d", two=2)
                            sq = p1.tile([128, 512], F32, name="sq",
                                         tag="sq", bufs=2)
                            nc.scalar.square(out=sq, in_=ps)
                            ssum = p1.tile([128, 8], F32, name="ssum",
                                           tag="ssum", bufs=4)
                            nc.vector.tensor_reduce(
                                ssum, sq.rearrange("p (h d) -> p h d", h=8),
                                axis=mybir.AxisListType.X,
                                op=mybir.AluOpType.add)
                            srt = p1.tile([128, 8], F32, name="srt", tag="srt",
                                          bufs=4)
                            nc.scalar.activation(
                                srt, ssum, mybir.ActivationFunctionType.Sqrt,
                                bias=epsc, scale=1.0 / 64.0)
                            rst = p1.tile([128, 8], F32, name="rst", tag="rst",
                                          bufs=4)
                            nc.vector.reciprocal(out=rst, in_=srt)
                            # rot = [t1_0 + u_1 | t1_1 - u_0]  (Pool)
                            rot = p1.tile([128, 8, 2, 32], F16,
                                          name=f"rot{which}",
                                          tag=f"rot{which}", bufs=2)
                            nc.gpsimd.tensor_add(
                                out=rot[:, :, 0, :], in0=t1[:, :, 0, :],
                                in1=u[:, :, 1, :])
                            nc.gpsimd.tensor_sub(
                                out=rot[:, :, 1, :], in0=t1[:, :, 1, :],
                                in1=u[:, :, 0, :])
                            # apply 1/rms (Pool), then DMA-transpose
                            rstb = rst.rearrange(
                                "p (h o) -> p h o", o=1).rearrange(
                                "p h (o d) -> p h o d", o=1).to_broadcast(
                                (128, 8, 2, 32))
                            rot2 = p1.tile([128, 8, 2, 32], F16,
                                           name=f"rr{which}",
                                           tag=f"rr{which}", bufs=2)
                            nc.gpsimd.tensor_mul(out=rot2, in0=rot, in1=rstb)
                            dstT = qT if which == "q" else kT
                            nc.sync.dma_start_transpose(
                                out=dstT[:, :, tg * 128:(tg + 1) * 128],
                                in_=rot2.rearrange("p h two d -> p (h two d)"))

                yT_of = {}
                pending = [None]

                def p2_gen(qc):
                    yT = p2.tile([128, 4, 512], F16, name="yT", tag="yT",
                                 bufs=3)
                    yT_of[qc] = yT
                    for hp in range(4):
                        pair = (2 * hp, 2 * hp + 1)
                        kts = list(range(4 * qc + 4))
                        lag = 3 if len(kts) > 3 else 2
                        pv = {}
                        for h in pair:
                            pv[h] = p1ps.tile([65, 512], F32, name="pv",
                                              tag="pv", bufs=2)
                        pt_live = {}

                        def emit_pv(kt, idx, pv=pv, pair=pair, kts=kts,
                                    qc=qc):
                            m = kt - 4 * qc
                            e0 = 128 * m if m > 0 else 0
                            pt = pt_live.pop(kt)
                            for si, h in enumerate(pair):
                                nc.tensor.matmul(
                                    pv[h][:, e0:512], vsb[kt][:, h, :],
                                    pt[:, si, e0:512],
                                    start=(idx == 0),
                                    stop=(idx == len(kts) - 1))

                        for idx, kt in enumerate(kts):
                            m = kt - 4 * qc
                            a0 = 128 * m if m > 0 else 0
                            st2 = p1ps.tile([128, 2, 512], F32, name="st",
                                            tag="st", bufs=2)
                            for si in range(2):
                                b0 = 64 * si
                                nc.tensor.matmul(
                                    st2[:, si, a0:512],
                                    kT[b0:b0 + 64, hp,
                                       kt * 128:(kt + 1) * 128],
                                    qT[b0:b0 + 64, hp,
                                       qc * 512 + a0:(qc + 1) * 512],
                                    start=True, stop=True)
                            pt = p2.tile([128, 2, 512], F16, name="pt",
                                         tag="pt", bufs=8)
                            nc.scalar.activation(
                                pt[:, :, a0:512], st2[:, :, a0:512],
                                mybir.ActivationFunctionType.Exp,
                                scale=SCALE)
                            if m >= 0:
                                w0 = 128 * m
                                for si in range(2):
                                    nc.vector.tensor_mul(
                                        out=pt[:, si, w0:w0 + 128],
                                        in0=pt[:, si, w0:w0 + 128],
                                        in1=tri01)
                            pt_live[kt] = pt
                            if idx == 3 and pending[0] is not None:
                                pending[0]()
                                pending[0] = None
                            if idx >= lag:
                                emit_pv(kts[idx - lag], idx - lag)
                            yield
                        for j in range(lag, 0, -1):
                            emit_pv(kts[-j], len(kts) - j)
                        if pending[0] is not None:
                            pending[0]()
                            pending[0] = None

                        # reciprocals now (DVE starts while the next pair's
                        # scores stream); broadcast + normalize deferred
                        # into the next pair's loop
                        recs = {}
                        for si, h in enumerate(pair):
                            rec = p2.tile([1, 512], F16, name="rec",
                                          tag="rec", bufs=4)
                            with nc.allow_low_precision(
                                    reason="softmax denom recip fp16"):
                                nc.vector.reciprocal(
                                    out=rec, in_=pv[h][64:65, :])
                            recs[h] = rec

                        def normalize(pv=pv, pair=pair, hp=hp, yT=yT,
                                      recs=recs):
                            bc_ps = p1ps.tile([128, 512], F32, name="bcps",
                                              tag="qkvps", bufs=2)
                            for si, h in enumerate(pair):
                                b0 = 64 * si
                                nc.tensor.matmul(bc_ps[b0:b0 + 64, :],
                                                 ones64, recs[h],
                                                 start=True, stop=True)
                            bc = p2.tile([128, 512], F16, name="bc", tag="bc",
                                         bufs=3)
                            nc.vector.tensor_copy(out=bc, in_=bc_ps)
                            for si, h in enumerate(pair):
                                b0 = 64 * si
                                nc.vector.tensor_mul(
                                    out=yT[b0:b0 + 64, hp, :],
                                    in0=pv[h][0:64, :], in1=bc[b0:b0 + 64, :])

                        pending[0] = normalize

                def proj_gen(qc):
                    yT = yT_of[qc]
                    for tsub in range(4):
                        for jc in range(2):
                            yield
                            pr = p1ps.tile([128, 512], F32, name="pr",
                                           tag="qkvps", bufs=2)
                            for ft in range(4):
                                nc.tensor.matmul(
                                    pr,
                                    yT[:, ft, tsub * 128:(tsub + 1) * 128],
                                    wp_sb[:, ft, jc * 512:(jc + 1) * 512],
                                    start=(ft == 0), stop=(ft == 3))
                            osb = p2.tile([128, 512], F16, name="osb",
                                          tag="osb", bufs=4)
                            nc.vector.tensor_copy(out=osb, in_=pr)
                            r0 = qc * 512 + tsub * 128
                            nc.sync.dma_start(
                                out=out_d[r0:r0 + 128,
                                          jc * 512:(jc + 1) * 512],
                                in_=osb)

                # software pipeline: start q,k of chunk 0; interleave the
                # v units + next chunk's QKV + previous chunk's projection
                # into each attention chunk's kt-step stream.
                g0 = p1_gen(0, xtiles=(x0h, x0l), use_st=True)
                for _ in range(9):      # all q and k units
                    next(g0)
                fill_counts = {0: 4 + 13, 1: 13 + 8, 2: 13, 3: 16}
                for qc in range(NQ):
                    chain = []
                    if qc == 0:
                        chain.append(g0)
                    if qc + 1 < NQ:
                        chain.append(p1_gen(qc + 1))
                    if qc == 1:
                        chain.append(proj_gen(0))
                    if qc == 3:
                        chain.append(proj_gen(1))
                        chain.append(proj_gen(2))
                    if not wp_loaded[0]:
                        nc.sync.dma_start(
                            out=wp_sb,
                            in_=wp_d.rearrange("(c p) j -> p c j", p=128))
                        wp_loaded[0] = True
                    steps = 4 * (4 * qc + 4)
                    nfill = fill_counts[qc]
                    acc = [0.0]
                    rate = nfill / steps

                    def fire():
                        while chain:
                            try:
                                next(chain[0])
                                return
                            except StopIteration:
                                chain.pop(0)

                    if qc == 0:
                        # fill the transpose-latency hole before step 0:
                        # v units + first next-chunk QKV units run on PE
                        # while chunk-0 q/k drain through DVE/Pool/DMA
                        for _ in range(6):
                            fire()
                    i = 0
                    for _ in p2_gen(qc):
                        i += 1
                        acc[0] += rate
                        while acc[0] >= 1.0:
                            acc[0] -= 1.0
                            fire()
                    while chain:
                        fire()
                        if not chain:
                            break
                if pending[0] is not None:
                    pending[0]()
                    pending[0] = None
                for _ in proj_gen(NQ - 1):
                    pass

    _legalize_waits(nc)
    return nc


def _host_tables():
    inv_freq = 1.0 / (10000.0 ** (np.arange(0, D, 2, dtype=np.float32) / D))
    t = np.arange(T, dtype=np.float32)
    freqs = np.outer(t, inv_freq).astype(np.float32)      # (T, 32)
    c = np.cos(freqs)
    sn = np.sin(freqs)
    cs16 = np.concatenate([c, c, sn, sn], axis=1).astype(np.float16)
    p = np.arange(128)[:, None]
    f = np.arange(128)[None, :]
    tri = (p <= f).astype(np.float16)                      # (128, 128)
    return cs16, tri


def _hilo(a):
    hi = a.astype(ml_dtypes.float8_e4m3)
    lo = (a - hi.astype(np.float32)).astype(ml_dtypes.float8_e4m3)
    return hi, lo


_CACHE = {}


def kernel(x, v1, wq, wk, wv, wproj, lamb):
    x = np.asarray(x, dtype=np.float32)
    v1 = np.asarray(v1, dtype=np.float32)
    wq = np.asarray(wq, dtype=np.float32)
    wk = np.asarray(wk, dtype=np.float32)
    wv = np.asarray(wv, dtype=np.float32)
    wproj = np.asarray(wproj, dtype=np.float32)
    lam = float(np.asarray(lamb))

    csn, tri = _host_tables()

    key = lam
    if key not in _CACHE:
        _CACHE[key] = _build(lam)
    nc = _CACHE[key]

    in_maps = []
    for core in range(8):
        b, hg = core // 2, core % 2
        sl = slice(hg * HG, (hg + 1) * HG)
        xh, xl = _hilo(np.ascontiguousarray(x[b].T))
        m = {
            "xTh": xh,
            "xTl": xl,
            "v1h": np.ascontiguousarray(
                (lam * v1[b][:, sl]).astype(np.float16)),
            "wpT": np.ascontiguousarray(wproj[:, sl].T.astype(np.float16)),
            "csn": csn,
            "tri01": tri,
        }
        for wn, w in (("q", wq), ("k", wk), ("v", wv)):
            wh, wl = _hilo(np.ascontiguousarray(w[sl, :].T) * 16.0)
            m[f"w{wn}h"] = wh
            m[f"w{wn}l"] = wl
        in_maps.append(m)

    res = bass_utils.run_bass_kernel_spmd(nc, in_maps, core_ids=list(range(8)))
    y = np.empty((B, T, C), dtype=np.float32)
    for b in range(B):
        y[b] = (res.results[2 * b]["out"].astype(np.float32)
                + res.results[2 * b + 1]["out"].astype(np.float32))
    return (y, v1)
